# revision 6
# baseline (speedup 1.0000x reference)
"""Distributed Trainium2 kernel for nn_ContrastiveLoss (SimCLR InfoNCE loss).

Math (matches the JAX reference):
    cos = (z/||z||) @ (z/||z||)^T          # [N, N], N=8192, D=1024
    logits = cos / T  (T=0.1), diag masked to -inf (exp -> 0)
    nll_i = -logits[i, (i+N/2) mod N] + log(sum_j exp(logits[i, j]))
    out = mean(nll)

Sharding: rows of z are split across 8 NeuronCores (1024 rows each).
Each core gets a column-ROLLED copy of normalized z^T (zr_c = zhat.T
rolled left by c*1024 columns) so that the self-diagonal block and the
positive-pair block land at *core-independent* static column offsets —
all 8 cores run the identical program on different data.

Per core: zhat_slab^T @ zhat^T_rolled via 1024 accumulating float32r
matmuls (PE, fp22 precision, full rate), fused exp+row-sum on the
Scalar engine (accum_out), diagonal extraction via identity-mask
tensor_tensor_reduce on the Vector engine. Row nll vectors are DMA'd
out; the host computes the final mean.
"""

import numpy as np

N, D = 8192, 1024
NCORES = 8
ROWS = N // NCORES      # 1024 rows per core
MT = ROWS // 128        # 8 m-tiles of 128 rows
KT = D // 128           # 8 k-tiles (contraction)
NTILE = 512             # moving-dim tile (one PSUM bank of fp32)
NT = N // NTILE         # 16 n-tiles
TEMP_INV = 10.0         # 1/temperature


def _import_concourse():
    import sys
    try:
        import concourse.bass  # noqa: F401
    except ImportError:
        for p in ("/root/.axon_site/_ro/trn_rl_repo", "/opt/trn_rl_repo"):
            if p not in sys.path:
                sys.path.insert(0, p)
        import concourse.bass  # noqa: F401


def build_program():
    _import_concourse()
    import concourse.mybir as mybir
    import concourse.tile as tile
    from concourse import bacc
    from concourse.masks import make_identity

    f32 = mybir.dt.float32
    f32r = mybir.dt.float32r
    Act = mybir.ActivationFunctionType
    Alu = mybir.AluOpType

    nc = bacc.Bacc()
    zr = nc.declare_dram_parameter("zr", [D, N], f32r, isOutput=False)
    out = nc.declare_dram_parameter("out", [128, MT], f32, isOutput=True)

    # zr[d, j]: d = k*128 + p (partition p, k-tile k); j = global free col
    zr_pkn = zr.rearrange("(k p) n -> p k n", p=128)
    zr_lhs = zr.rearrange("(k p) (m f) -> p k m f", p=128, f=128)

    with tile.TileContext(nc) as tc:
        with (
            tc.tile_pool(name="consts", bufs=1) as consts,
            tc.tile_pool(name="lhsp", bufs=1) as lhsp,
            tc.tile_pool(name="rhsp", bufs=3) as rhsp,
            tc.tile_pool(name="psump", bufs=8, space="PSUM") as psump,
            tc.tile_pool(name="escp", bufs=3) as escp,
            tc.tile_pool(name="smallp", bufs=4) as smallp,
            tc.tile_pool(name="accp", bufs=1) as accp,
        ):
            ident = consts.tile([128, 128], f32)
            make_identity(nc, ident)
            # DVE warmup read of ident: advances DVE's observed GpSimd
            # vector-clock so later tensor_mul(psum, ident) ops carry only
            # one sync wait (walrus: DVE TensorTensor allows a single wait).
            identw = consts.tile([128, 1], f32)
            nc.vector.reduce_max(
                out=identw, in_=ident, axis=mybir.AxisListType.X
            )

            # resident lhsT slab: zr[:, :1024] as [p, k, m, f]
            lhs_t = lhsp.tile([128, KT, MT, 128], f32r)
            for k in range(KT):
                nc.sync.dma_start(out=lhs_t[:, k], in_=zr_lhs[:, k, 0:MT])

            acc = accp.tile([128, MT * NT], f32)   # per-(m, n) exp row sums
            dcol = accp.tile([128, MT], f32)       # self-diag cos values
            pcol = accp.tile([128, MT], f32)       # 10 * positive-pair cos

            for n in range(NT):
                rhs_t = rhsp.tile([128, KT, NTILE], f32r)
                for k in range(KT):
                    nc.sync.dma_start(
                        out=rhs_t[:, k],
                        in_=zr_pkn[:, k, n * NTILE : (n + 1) * NTILE],
                    )
                for m in range(MT):
                    ps = psump.tile([128, NTILE], f32)
                    for k in range(KT):
                        nc.tensor.matmul(
                            ps,
                            lhsT=lhs_t[:, k, m, :],
                            rhs=rhs_t[:, k, :],
                            start=(k == 0),
                            stop=(k == KT - 1),
                        )
                    if n == m // 4:
                        # self-diagonal block: local col m*128 + p
                        off = (m % 4) * 128
                        dtmp = smallp.tile([128, 128], f32, tag="blk")
                        nc.vector.tensor_mul(
                            out=dtmp, in0=ps[:, off : off + 128], in1=ident
                        )
                        nc.vector.reduce_sum(
                            out=dcol[:, m : m + 1], in_=dtmp,
                            axis=mybir.AxisListType.X,
                        )
                    if n == MT + m // 4:
                        # positive-pair block: local col 4096 + m*128 + p
                        off = (m % 4) * 128
                        ptmp = smallp.tile([128, 128], f32, tag="blk")
                        nc.vector.tensor_mul(
                            out=ptmp, in0=ps[:, off : off + 128], in1=ident
                        )
                        nc.vector.reduce_sum(
                            out=pcol[:, m : m + 1], in_=ptmp,
                            axis=mybir.AxisListType.X,
                        )
                    # exp(10 * cos) with fused row-sum into acc[:, m*NT+n]
                    esc = escp.tile([128, NTILE], f32)
                    idx = m * NT + n
                    nc.scalar.activation(
                        out=esc,
                        in_=ps,
                        func=Act.Exp,
                        scale=TEMP_INV,
                        accum_out=acc[:, idx : idx + 1],
                    )

            outt = accp.tile([128, MT], f32)
            for m in range(MT):
                s = smallp.tile([128, 1], f32, tag="sc")
                nc.vector.reduce_sum(
                    out=s, in_=acc[:, m * NT : (m + 1) * NT],
                    axis=mybir.AxisListType.X,
                )
                ed = smallp.tile([128, 1], f32, tag="sc")
                nc.scalar.activation(
                    out=ed, in_=dcol[:, m : m + 1], func=Act.Exp, scale=TEMP_INV
                )
                s2 = smallp.tile([128, 1], f32, tag="sc")
                nc.vector.tensor_sub(out=s2, in0=s, in1=ed)
                lse = smallp.tile([128, 1], f32, tag="sc")
                nc.scalar.activation(out=lse, in_=s2, func=Act.Ln)
                # nll = lse - 10 * pos_cos  (pcol holds raw cos values)
                nc.vector.tensor_scalar(
                    out=outt[:, m : m + 1],
                    in0=pcol[:, m : m + 1],
                    scalar1=-TEMP_INV,
                    scalar2=lse,
                    op0=Alu.mult,
                    op1=Alu.add,
                )
            nc.sync.dma_start(out=out[:, :], in_=outt)
    nc.finalize()
    return nc


def make_in_maps(z: np.ndarray) -> list[dict]:
    z = np.ascontiguousarray(np.asarray(z, dtype=np.float32))
    norms = np.sqrt((z.astype(np.float64) ** 2).sum(axis=-1))
    zn = (z / norms[:, None]).astype(np.float32)
    zt = np.ascontiguousarray(zn.T)  # [D, N]
    in_maps = []
    for c in range(NCORES):
        s = c * ROWS
        if s == 0:
            zr = zt
        else:
            zr = np.ascontiguousarray(
                np.concatenate([zt[:, s:], zt[:, :s]], axis=1)
            )
        in_maps.append({"zr": zr})
    return in_maps


def assemble(results: list[dict]) -> np.ndarray:
    # results[c]["out"][p, m] = nll of global row c*1024 + m*128 + p
    nll = np.stack([np.asarray(r["out"], np.float32) for r in results])  # [c,p,m]
    nll = nll.transpose(0, 2, 1).reshape(-1)  # global row order
    return np.float32(nll.mean())


def kernel(z: np.ndarray) -> np.ndarray:
    _import_concourse()
    from concourse.bass_utils import run_bass_kernel_spmd

    nc = build_program()
    in_maps = make_in_maps(z)
    res = run_bass_kernel_spmd(nc, in_maps, core_ids=list(range(NCORES)))
    return assemble(res.results)
